# revision 10
# baseline (speedup 1.0000x reference)
"""Trainium2 Bass kernel for nn_CrossLayerV2 (MoE low-rank bilinear cross layer).

Computes, for x0,x [B,D], U [E,D,R], V [E,R,D], C [E,R,R], b [E,D], Wg [E,D], bg [E]:
    g    = softmax(x @ Wg.T + bg, axis=1)                 # [B, E]
    xvc  = einsum('bd,eds->bes', x, W1)  with W1[e] = V[e].T @ C[e]   (host-fused)
    out  = x0 * einsum('be,bes,eds->bd', g, xvc, U) + g @ b + x * g.sum(1, keepdims=True)
Note g.sum(1) == 1 (softmax), so the last term is exactly x.

Strategy: data-parallel over batch across 8 NeuronCores (params replicated).
fp8(e4m3, max 240) everywhere on the matmul path with DoubleRow perf mode
(2 k-tiles of 128 per instruction, 0.5 cyc/row = 2x bf16 throughput):

  xvcT[es,b]  = (2^8 W1)^T x^T          DR fp8 matmuls, es-chunked [128,512] psum
  logT[e,b]   = (2^6 Wg)^T x^T          1 DR matmul per 512-row block
  pT[e,b]     = exp(logT*2^-6 + bg - 3ln2)   unnormalized gate / 8, fp8 SBUF (ACT)
  S'[1,b]     = ones^T pT               PE partition-reduce; *2^14 on psum->sbuf copy
  s4[b,1]     = transpose chunks of S'  PE; r4 = 1/s4 = 2^-14/S' per-row scalar (DVE)
  grep[q,b]   = sel_c^T pT              PE replicate matmul (expert row -> 64 ranks)
  gxvc[q,b]   = xvcT * grep             DVE/GP elementwise, fp8 out
  p_o[b,d]    = (2^6 Upk)^T-contraction of gxvc   DR fp8 matmuls, psum = 2^14*Sum'
  out         = (p_o * r4[row]) * x0 + x          stt (DVE/GP/ACT-assisted) + add

The 1/S softmax normalization and the 2^-14 param prescale cancel through the
final per-row scalar, so the gate is never normalized on its own.
"""

import sys

for _p in ("/opt/trn_rl_repo", "/opt/pypackages"):
    if _p not in sys.path:
        sys.path.append(_p)

from contextlib import ExitStack

import ml_dtypes
import numpy as np

import concourse.bass as bass
import concourse.tile as tile
from concourse import mybir

BF16 = mybir.dt.bfloat16
F32 = mybir.dt.float32
FP8 = mybir.dt.float8e4
NPBF16 = ml_dtypes.bfloat16
NPFP8 = ml_dtypes.float8_e4m3
DR = mybir.MatmulPerfMode.DoubleRow

B, D, R, E = 16384, 512, 64, 8
NCORES = 8
BL = B // NCORES          # rows per core (2048)
P = 128                   # partitions
NSUB = 4                  # subtiles per block
BLOCK = NSUB * P          # rows per block (512)
NBLK = BL // BLOCK        # blocks per core (4)
KC = D // P               # contraction chunks of 128 (4)
KP = KC // 2              # DoubleRow k-pairs (2)
NCH = 4                   # es chunks of 128
ES = E * R                # expert-packed width (512)
EPAD = 32                 # gate logit columns padded for DoubleRow ldweights

SC_W1 = 2.0 ** 8          # host prescale of W1 (fp8 range)
SC_WG = 2.0 ** 6          # host prescale of Wg
SC_UT = 2.0 ** 6          # host prescale of U
SC_PS = 2.0 ** 14         # = SC_W1 * SC_UT, folded into r4 via the S' copy
EXP_BIAS = -3.0 * float(np.log(2.0))  # exp outputs p/8 so gxvc stays in fp8 range

# engine assignment knobs (tuned from traces). GpSimd cannot touch PSUM, so
# every psum-draining op lives on DVE or ACT; GpSimd gets SBUF-only work
# (the 16-bit stt after an ACT psum->sbuf copy, and the residual adds).
GATING_ENG = ("dve", "dve", "dve", "dve")     # per es-chunk (psum: DVE only)
GREP_COPY_ENG = ("act", "dve", "act", "dve")  # per es-chunk psum->sbuf copy
STT_MODE = ("act2x", "act2x", "act2x", "dve")  # per subtile: act2x|dve
ADD_ENG = "gp_batch"                          # gp_batch|dve_batch|split


def _kernel_body(tc, out_d, xt_d, xr_d, x0_d, w1_d, wg_d, ut_d, sel_d, bias_d,
                 bexp_d, zero_b):
    nc = tc.nc

    with ExitStack() as ctx:
        const = ctx.enter_context(tc.tile_pool(name="const", bufs=1))
        gx = ctx.enter_context(tc.tile_pool(name="gx", bufs=2))
        gwork = ctx.enter_context(tc.tile_pool(name="gwork", bufs=2))
        pwork = ctx.enter_context(tc.tile_pool(name="pwork", bufs=3))
        outp = ctx.enter_context(tc.tile_pool(name="outp", bufs=2))
        ps_s = ctx.enter_context(tc.tile_pool(name="ps_s", bufs=1, space="PSUM"))
        ps_s4 = ctx.enter_context(tc.tile_pool(name="ps_s4", bufs=1, space="PSUM"))
        ps_xvc = ctx.enter_context(tc.tile_pool(name="ps_xvc", bufs=2, space="PSUM"))
        ps_grep = ctx.enter_context(tc.tile_pool(name="ps_grep", bufs=2, space="PSUM"))
        ps_out = ctx.enter_context(tc.tile_pool(name="ps_out", bufs=2, space="PSUM"))

        # --- resident inputs / params (all DMAs issued up-front) ---
        ident1 = const.tile([1, 1], F32)
        nc.vector.memset(ident1, 1.0)
        ones8 = const.tile([E, 1], FP8)
        nc.gpsimd.memset(ones8, 1.0)

        wg_sb = const.tile([P, KP, 2, EPAD], FP8)
        nc.sync.dma_start(wg_sb, wg_d)
        xt_sb = const.tile([P, KC, BL], FP8)
        nc.sync.dma_start(xt_sb[:, :, :BLOCK], xt_d[:, :, :BLOCK])
        w1_sb = const.tile([P, KP, 2, NCH, P], FP8)
        nc.sync.dma_start(w1_sb, w1_d)
        ut_sb = const.tile([P, KP, 2, D], FP8)
        nc.sync.dma_start(ut_sb, ut_d)
        nc.sync.dma_start(xt_sb[:, :, BLOCK:], xt_d[:, :, BLOCK:])

        sel_sb = const.tile([E, NCH, P], FP8)
        nc.scalar.dma_start(sel_sb, sel_d)
        bias_sb = const.tile([E, 1], F32)
        nc.scalar.dma_start(bias_sb, bias_d)
        if not zero_b:
            bexp_sb = const.tile([E, D], BF16)
            nc.scalar.dma_start(bexp_sb, bexp_d)

        x0_sb = const.tile([P, NBLK, NSUB, D], FP8)
        nc.gpsimd.dma_start(x0_sb[:, :2], x0_d[:, :2])
        xr_sb = const.tile([P, NBLK, NSUB, D], BF16)
        nc.gpsimd.dma_start(xr_sb[:, :2], xr_d[:, :2])
        nc.gpsimd.dma_start(x0_sb[:, 2:], x0_d[:, 2:])
        nc.gpsimd.dma_start(xr_sb[:, 2:], xr_d[:, 2:])

        def gate_head(blk):
            """Transposed gate logits + exp for one block; returns (expT, r4)."""
            bsl = slice(blk * BLOCK, (blk + 1) * BLOCK)
            # p_log and pS share one psum slot (same tag): pS is written only
            # after expT has fully read p_log, so the ring-1 WAR is natural.
            p_log = ps_s.tile([EPAD, BLOCK], F32, tag="S")
            for i in range(KP):
                nc.tensor.matmul(p_log, wg_sb[:, i], xt_sb[:, 2 * i:2 * i + 2, bsl],
                                 start=(i == 0), stop=(i == KP - 1), perf_mode=DR)
            expT = gwork.tile([E, BLOCK], FP8, tag="expT")
            nc.scalar.activation(expT, p_log[:E], mybir.ActivationFunctionType.Exp,
                                 bias=bias_sb[:, :], scale=1.0 / SC_WG)
            pS = ps_s.tile([1, BLOCK], F32, tag="S")
            nc.tensor.matmul(pS, ones8, expT)
            S_sb = gwork.tile([1, BLOCK], F32, tag="S_sb")
            nc.scalar.activation(S_sb, pS, mybir.ActivationFunctionType.Copy,
                                 scale=SC_PS)
            ps4 = ps_s4.tile([P, NSUB], F32, tag="s4")
            for s in range(NSUB):
                nc.tensor.matmul(ps4[:, s:s + 1], S_sb[:, s * P:(s + 1) * P],
                                 ident1, is_transpose=True)
            r4 = gwork.tile([P, NSUB], F32, tag="r4")
            nc.vector.reciprocal(r4, ps4)
            return expT, r4

        def chunk_phase(blk, expT, gxvcs, c):
            """grep + xvcT matmuls + gating multiply for es-chunk c."""
            bsl = slice(blk * BLOCK, (blk + 1) * BLOCK)
            i, j = divmod(c, 2)
            pg = ps_grep.tile([P, BLOCK], F32, tag="grep")
            nc.tensor.matmul(pg, sel_sb[:, c, :], expT)
            grep8 = pwork.tile([P, BLOCK], FP8, tag="grep8")
            if GREP_COPY_ENG[c] == "act":
                nc.scalar.copy(grep8, pg)
            else:
                nc.vector.tensor_copy(grep8, pg)
            px = ps_xvc.tile([P, BLOCK], F32, tag="xvc")
            for i2 in range(KP):
                nc.tensor.matmul(px, w1_sb[:, i2, :, c, :],
                                 xt_sb[:, 2 * i2:2 * i2 + 2, bsl],
                                 start=(i2 == 0), stop=(i2 == KP - 1), perf_mode=DR)
            eng = nc.vector if GATING_ENG[c] == "dve" else nc.gpsimd
            eng.tensor_mul(gxvcs[:, i, j, :], px, grep8)

        def out_phase(blk, state):
            """Second contraction + residual epilogue + store for one block."""
            gxvcs, r4, expT_bf = state
            out_t = outp.tile([P, NSUB, D], BF16, tag="o")
            for s in range(NSUB):
                ssl = slice(s * P, (s + 1) * P)
                po = ps_out.tile([P, D], F32, tag="po")
                for i2 in range(KP):
                    nc.tensor.matmul(po, gxvcs[:, i2, :, ssl], ut_sb[:, i2],
                                     start=(i2 == 0),
                                     stop=(zero_b and i2 == KP - 1), perf_mode=DR)
                if not zero_b:
                    nc.tensor.matmul(po, expT_bf[:, ssl], bexp_sb,
                                     start=False, stop=True)
                x0s = x0_sb[:, blk, s, :]
                rs = r4[:, s:s + 1]
                mode = STT_MODE[s]
                if mode == "act2x":
                    # ACT drains psum to bf16; DVE does the cheap 16-bit
                    # (po*r)*x0 fused multiply (Pool lacks TensorScalarPtr).
                    po_bf = pwork.tile([P, D], BF16, tag="pobf")
                    nc.scalar.copy(po_bf, po)
                    nc.vector.scalar_tensor_tensor(
                        out_t[:, s, :], po_bf, rs, x0s,
                        op0=mybir.AluOpType.mult, op1=mybir.AluOpType.mult)
                else:
                    nc.vector.scalar_tensor_tensor(
                        out_t[:, s, :], po, rs, x0s,
                        op0=mybir.AluOpType.mult, op1=mybir.AluOpType.mult)
            if ADD_ENG == "gp_batch":
                nc.gpsimd.tensor_add(out_t, out_t, xr_sb[:, blk])
            elif ADD_ENG == "dve_batch":
                nc.vector.tensor_add(out_t, out_t, xr_sb[:, blk])
            else:
                nc.vector.tensor_add(out_t[:, :2], out_t[:, :2], xr_sb[:, blk, :2])
                nc.gpsimd.tensor_add(out_t[:, 2:], out_t[:, 2:], xr_sb[:, blk, 2:])
            nc.sync.dma_start(out_d[:, blk], out_t)

        prev = None
        for blk in range(NBLK):
            expT, r4 = gate_head(blk)
            expT_bf = None
            if not zero_b:
                expT_bf = gwork.tile([E, BLOCK], BF16, tag="expT_bf")
                nc.scalar.copy(expT_bf, expT)
            gxvcs = gx.tile([P, KP, 2, BLOCK], FP8, tag="gx")
            chunk_phase(blk, expT, gxvcs, 0)
            chunk_phase(blk, expT, gxvcs, 1)
            if prev is not None:
                out_phase(blk - 1, prev)
            chunk_phase(blk, expT, gxvcs, 2)
            chunk_phase(blk, expT, gxvcs, 3)
            prev = (gxvcs, r4, expT_bf)
        out_phase(NBLK - 1, prev)


def _split_excess_waits(nc: bass.Bass, cap: int = 1) -> None:
    """Walrus's per-instruction sync encoders take few wait slots (the TT
    struct rejects 2+). Move extra semaphore waits onto preceding NoOps on
    the same engine; engine program order preserves the semantics."""
    counter = [0]
    for f in nc.m.functions:
        for blk in f.blocks:
            il = blk.instructions
            out = []
            changed = False
            for ins in il:
                si = ins.sync_info
                if si is not None and len(si.on_wait) > cap:
                    extra = list(si.on_wait[:-cap]) if cap else list(si.on_wait)
                    keep = list(si.on_wait[-cap:]) if cap else []
                    for w in extra:
                        nop = mybir.InstNoOp(name=f"NOPW-{counter[0]}")
                        counter[0] += 1
                        nop.engine = ins.engine
                        nop.sync_info = mybir.SyncInfo(on_wait=[w], on_update=[])
                        nc.register_instruction(nop)
                        out.append(nop)
                    ins.sync_info = mybir.SyncInfo(on_wait=keep,
                                                   on_update=list(si.on_update))
                    changed = True
                out.append(ins)
            if changed:
                blk.instructions = out


def build_module(zero_b: bool = True) -> bass.Bass:
    nc = bass.Bass("TRN2", target_bir_lowering=False, debug=False)
    xt_d = nc.dram_tensor("xt8", [P, KC, BL], FP8, kind="ExternalInput").ap()
    xr_d = nc.dram_tensor("xr", [P, NBLK, NSUB, D], BF16, kind="ExternalInput").ap()
    x0_d = nc.dram_tensor("x0q", [P, NBLK, NSUB, D], FP8, kind="ExternalInput").ap()
    w1_d = nc.dram_tensor("w1", [P, KP, 2, NCH, P], FP8, kind="ExternalInput").ap()
    wg_d = nc.dram_tensor("wg", [P, KP, 2, EPAD], FP8, kind="ExternalInput").ap()
    ut_d = nc.dram_tensor("ut", [P, KP, 2, D], FP8, kind="ExternalInput").ap()
    sel_d = nc.dram_tensor("sel", [E, NCH, P], FP8, kind="ExternalInput").ap()
    bias_d = nc.dram_tensor("gbias", [E, 1], F32, kind="ExternalInput").ap()
    bexp_d = nc.dram_tensor("bexp", [E, D], BF16, kind="ExternalInput").ap()
    out_d = nc.dram_tensor("out", [P, NBLK, NSUB, D], BF16,
                           kind="ExternalOutput").ap()
    with tile.TileContext(nc) as tc:
        _kernel_body(tc, out_d, xt_d, xr_d, x0_d, w1_d, wg_d, ut_d, sel_d,
                     bias_d, bexp_d, zero_b)
    _split_excess_waits(nc)
    return nc


_NC_CACHE: dict = {}


def _get_module(zero_b: bool = True) -> bass.Bass:
    if zero_b not in _NC_CACHE:
        _NC_CACHE[zero_b] = build_module(zero_b)
    return _NC_CACHE[zero_b]


def make_in_maps(x0, x, U, V, C, b, Wg, bg):
    x0 = np.asarray(x0, dtype=np.float32)
    x = np.asarray(x, dtype=np.float32)
    U = np.asarray(U, dtype=np.float32)
    V = np.asarray(V, dtype=np.float32)
    C = np.asarray(C, dtype=np.float32)
    b = np.asarray(b, dtype=np.float32)
    Wg = np.asarray(Wg, dtype=np.float32)
    bg = np.asarray(bg, dtype=np.float32)

    # W1[e] = V[e].T @ C[e] -> [D, ES]; packed for DoubleRow lhsT slices
    w1t = np.einsum("erd,ers->eds", V, C).transpose(1, 0, 2).reshape(D, ES)
    w1 = (w1t * SC_W1).reshape(KP, 2, P, NCH, P).transpose(2, 0, 1, 3, 4)
    w1 = np.ascontiguousarray(w1).astype(NPFP8)
    wg_p = (Wg.T * SC_WG).reshape(KP, 2, P, E).transpose(2, 0, 1, 3)
    wg = np.zeros((P, KP, 2, EPAD), np.float32)
    wg[..., :E] = wg_p
    wg = wg.astype(NPFP8)
    # Upk[e*R+s, d] = U[e,d,s]; packed for DoubleRow rhs slices
    ut = (U.transpose(0, 2, 1).reshape(ES, D) * SC_UT)
    ut = ut.reshape(KP, 2, P, D).transpose(2, 0, 1, 3)
    ut = np.ascontiguousarray(ut).astype(NPFP8)
    # replicate matrices: chunk c covers experts 2c (ranks 0-63), 2c+1 (64-127)
    sel = np.zeros((E, NCH, P), np.float32)
    for c in range(NCH):
        sel[2 * c, c, :R] = 1.0
        sel[2 * c + 1, c, R:] = 1.0
    sel = sel.astype(NPFP8)
    gbias = (bg.astype(np.float64) + EXP_BIAS).astype(np.float32).reshape(E, 1)
    bexp = (b * SC_PS).astype(NPBF16)

    x0_bf = x0.astype(NPFP8)
    x_bf = x.astype(NPBF16)
    xt_all = np.ascontiguousarray(x.T).astype(NPFP8)  # [D, B]

    in_maps = []
    for core in range(NCORES):
        sl = slice(core * BL, (core + 1) * BL)
        xt8 = np.ascontiguousarray(
            xt_all[:, sl].reshape(KC, P, BL).transpose(1, 0, 2))
        xr = np.ascontiguousarray(
            x_bf[sl].reshape(NBLK, NSUB, P, D).transpose(2, 0, 1, 3))
        x0q = np.ascontiguousarray(
            x0_bf[sl].reshape(NBLK, NSUB, P, D).transpose(2, 0, 1, 3))
        in_maps.append({
            "xt8": xt8, "xr": xr, "x0q": x0q,
            "w1": w1, "wg": wg, "ut": ut, "sel": sel, "gbias": gbias,
            "bexp": bexp,
        })
    return in_maps


def kernel(x0, x, U, V, C, b, Wg, bg, _trace=False, _trace_kwargs=None):
    from concourse.bass_utils import run_bass_kernel_spmd
    nc = _get_module(not np.any(np.asarray(b)))
    in_maps = make_in_maps(x0, x, U, V, C, b, Wg, bg)
    res = run_bass_kernel_spmd(nc, in_maps, list(range(NCORES)),
                               trace=_trace, **(_trace_kwargs or {}))
    outs = []
    for c in range(NCORES):
        o = np.asarray(res.results[c]["out"])  # [P, NBLK, NSUB, D] bf16
        outs.append(o.transpose(1, 2, 0, 3).reshape(BL, D))
    out = np.concatenate(outs, axis=0).astype(np.float32)
    if _trace:
        return out, res
    return out


if __name__ == "__main__":
    rng = np.random.default_rng(0)
    ins = {
        "x0": rng.standard_normal((B, D), dtype=np.float32),
        "x": rng.standard_normal((B, D), dtype=np.float32),
        "U": (rng.standard_normal((E, D, R)) * 0.02).astype(np.float32),
        "V": (rng.standard_normal((E, R, D)) * 0.02).astype(np.float32),
        "C": (rng.standard_normal((E, R, R)) * 0.02).astype(np.float32),
        "b": np.zeros((E, D), np.float32),
        "Wg": (rng.standard_normal((E, D)) * 0.02).astype(np.float32),
        "bg": np.zeros((E,), np.float32),
    }
    out = kernel(**ins)
    print("out", out.shape, out.dtype)


# revision 11
# speedup vs baseline: 1.0069x; 1.0069x over previous
"""Trainium2 Bass kernel for nn_CrossLayerV2 (MoE low-rank bilinear cross layer).

Computes, for x0,x [B,D], U [E,D,R], V [E,R,D], C [E,R,R], b [E,D], Wg [E,D], bg [E]:
    g    = softmax(x @ Wg.T + bg, axis=1)                 # [B, E]
    xvc  = einsum('bd,eds->bes', x, W1)  with W1[e] = V[e].T @ C[e]   (host-fused)
    out  = x0 * einsum('be,bes,eds->bd', g, xvc, U) + g @ b + x * g.sum(1, keepdims=True)
Note g.sum(1) == 1 (softmax), so the last term is exactly x.

Strategy: data-parallel over batch across 8 NeuronCores (params replicated).
fp8(e4m3, max 240) everywhere on the matmul path with DoubleRow perf mode
(2 k-tiles of 128 per instruction, 0.5 cyc/row = 2x bf16 throughput):

  xvcT[es,b]  = (2^8 W1)^T x^T          DR fp8 matmuls, es-chunked [128,512] psum
  logT[e,b]   = (2^6 Wg)^T x^T          1 DR matmul per 512-row block
  pT[e,b]     = exp(logT*2^-6 + bg - 3ln2)   unnormalized gate / 8, fp8 SBUF (ACT)
  S'[1,b]     = ones^T pT               PE partition-reduce; *2^14 on psum->sbuf copy
  s4[b,1]     = transpose chunks of S'  PE; r4 = 1/s4 = 2^-14/S' per-row scalar (DVE)
  grep[q,b]   = sel_c^T pT              PE replicate matmul (expert row -> 64 ranks)
  gxvc[q,b]   = xvcT * grep             DVE/GP elementwise, fp8 out
  p_o[b,d]    = (2^6 Upk)^T-contraction of gxvc   DR fp8 matmuls, psum = 2^14*Sum'
  out         = (p_o * r4[row]) * x0 + x          stt (DVE/GP/ACT-assisted) + add

The 1/S softmax normalization and the 2^-14 param prescale cancel through the
final per-row scalar, so the gate is never normalized on its own.
"""

import sys

for _p in ("/opt/trn_rl_repo", "/opt/pypackages"):
    if _p not in sys.path:
        sys.path.append(_p)

from contextlib import ExitStack

import ml_dtypes
import numpy as np

import concourse.bass as bass
import concourse.tile as tile
from concourse import mybir

BF16 = mybir.dt.bfloat16
F32 = mybir.dt.float32
FP8 = mybir.dt.float8e4
NPBF16 = ml_dtypes.bfloat16
NPFP8 = ml_dtypes.float8_e4m3
DR = mybir.MatmulPerfMode.DoubleRow

B, D, R, E = 16384, 512, 64, 8
NCORES = 8
BL = B // NCORES          # rows per core (2048)
P = 128                   # partitions
NSUB = 4                  # subtiles per block
BLOCK = NSUB * P          # rows per block (512)
NBLK = BL // BLOCK        # blocks per core (4)
KC = D // P               # contraction chunks of 128 (4)
KP = KC // 2              # DoubleRow k-pairs (2)
NCH = 4                   # es chunks of 128
ES = E * R                # expert-packed width (512)
EPAD = 32                 # gate logit columns padded for DoubleRow ldweights

SC_W1 = 2.0 ** 8          # host prescale of W1 (fp8 range)
SC_WG = 2.0 ** 6          # host prescale of Wg
SC_UT = 2.0 ** 6          # host prescale of U
SC_PS = 2.0 ** 14         # = SC_W1 * SC_UT, folded into r4 via the S' copy
EXP_BIAS = -3.0 * float(np.log(2.0))  # exp outputs p/8 so gxvc stays in fp8 range

# engine assignment knobs (tuned from traces). GpSimd cannot touch PSUM, so
# every psum-draining op lives on DVE or ACT; GpSimd gets SBUF-only work
# (the 16-bit stt after an ACT psum->sbuf copy, and the residual adds).
GATING_ENG = ("dve", "dve", "dve", "dve")     # per es-chunk (psum: DVE only)
GREP_COPY_ENG = ("act", "dve", "act", "dve")  # per es-chunk psum->sbuf copy
STT_MODE = ("dve", "dve", "dve", "dve")       # per subtile: act2x|dve
ADD_ENG = "gp_sub"                            # gp_sub|gp_batch|dve_batch|split


def _kernel_body(tc, out_d, xt_d, xr_d, x0_d, w1_d, wg_d, ut_d, sel_d, bias_d,
                 bexp_d, zero_b):
    nc = tc.nc

    with ExitStack() as ctx:
        const = ctx.enter_context(tc.tile_pool(name="const", bufs=1))
        gx = ctx.enter_context(tc.tile_pool(name="gx", bufs=2))
        gwork = ctx.enter_context(tc.tile_pool(name="gwork", bufs=3))
        pwork = ctx.enter_context(tc.tile_pool(name="pwork", bufs=3))
        outp = ctx.enter_context(tc.tile_pool(name="outp", bufs=2))
        ps_s = ctx.enter_context(tc.tile_pool(name="ps_s", bufs=1, space="PSUM"))
        ps_s4 = ctx.enter_context(tc.tile_pool(name="ps_s4", bufs=1, space="PSUM"))
        ps_xvc = ctx.enter_context(tc.tile_pool(name="ps_xvc", bufs=2, space="PSUM"))
        ps_grep = ctx.enter_context(tc.tile_pool(name="ps_grep", bufs=2, space="PSUM"))
        ps_out = ctx.enter_context(tc.tile_pool(name="ps_out", bufs=2, space="PSUM"))

        # --- resident inputs / params (all DMAs issued up-front) ---
        ident1 = const.tile([1, 1], F32)
        nc.vector.memset(ident1, 1.0)
        ones8 = const.tile([E, 1], FP8)
        nc.gpsimd.memset(ones8, 1.0)

        wg_sb = const.tile([P, KP, 2, EPAD], FP8)
        nc.sync.dma_start(wg_sb, wg_d)
        xt_sb = const.tile([P, KC, BL], FP8)
        nc.sync.dma_start(xt_sb[:, :, :BLOCK], xt_d[:, :, :BLOCK])
        w1_sb = const.tile([P, KP, 2, NCH, P], FP8)
        nc.sync.dma_start(w1_sb, w1_d)
        ut_sb = const.tile([P, KP, 2, D], FP8)
        nc.sync.dma_start(ut_sb, ut_d)
        nc.sync.dma_start(xt_sb[:, :, BLOCK:], xt_d[:, :, BLOCK:])

        sel_sb = const.tile([E, NCH, P], FP8)
        nc.scalar.dma_start(sel_sb, sel_d)
        bias_sb = const.tile([E, 1], F32)
        nc.scalar.dma_start(bias_sb, bias_d)
        if not zero_b:
            bexp_sb = const.tile([E, D], BF16)
            nc.scalar.dma_start(bexp_sb, bexp_d)

        x0_sb = const.tile([P, NBLK, NSUB, D], FP8)
        nc.gpsimd.dma_start(x0_sb[:, :2], x0_d[:, :2])
        xr_sb = const.tile([P, NBLK, NSUB, D], BF16)
        nc.gpsimd.dma_start(xr_sb[:, :2], xr_d[:, :2])
        nc.gpsimd.dma_start(x0_sb[:, 2:], x0_d[:, 2:])
        nc.gpsimd.dma_start(xr_sb[:, 2:], xr_d[:, 2:])

        def gate_head(blk):
            """Transposed gate logits + exp for one block; returns (expT, r4)."""
            bsl = slice(blk * BLOCK, (blk + 1) * BLOCK)
            # p_log and pS share one psum slot (same tag): pS is written only
            # after expT has fully read p_log, so the ring-1 WAR is natural.
            p_log = ps_s.tile([EPAD, BLOCK], F32, tag="S")
            for i in range(KP):
                nc.tensor.matmul(p_log, wg_sb[:, i], xt_sb[:, 2 * i:2 * i + 2, bsl],
                                 start=(i == 0), stop=(i == KP - 1), perf_mode=DR)
            expT = gwork.tile([E, BLOCK], FP8, tag="expT")
            nc.scalar.activation(expT, p_log[:E], mybir.ActivationFunctionType.Exp,
                                 bias=bias_sb[:, :], scale=1.0 / SC_WG)
            pS = ps_s.tile([1, BLOCK], F32, tag="S")
            nc.tensor.matmul(pS, ones8, expT)
            S_sb = gwork.tile([1, BLOCK], F32, tag="S_sb")
            nc.scalar.activation(S_sb, pS, mybir.ActivationFunctionType.Copy,
                                 scale=SC_PS)
            ps4 = ps_s4.tile([P, NSUB], F32, tag="s4")
            for s in range(NSUB):
                nc.tensor.matmul(ps4[:, s:s + 1], S_sb[:, s * P:(s + 1) * P],
                                 ident1, is_transpose=True)
            r4 = gwork.tile([P, NSUB], F32, tag="r4")
            nc.vector.reciprocal(r4, ps4)
            return expT, r4

        def chunk_phase(blk, expT, gxvcs, c):
            """grep + xvcT matmuls + gating multiply for es-chunk c."""
            bsl = slice(blk * BLOCK, (blk + 1) * BLOCK)
            i, j = divmod(c, 2)
            pg = ps_grep.tile([P, BLOCK], F32, tag="grep")
            nc.tensor.matmul(pg, sel_sb[:, c, :], expT)
            grep8 = pwork.tile([P, BLOCK], FP8, tag="grep8")
            if GREP_COPY_ENG[c] == "act":
                nc.scalar.copy(grep8, pg)
            else:
                nc.vector.tensor_copy(grep8, pg)
            px = ps_xvc.tile([P, BLOCK], F32, tag="xvc")
            for i2 in range(KP):
                nc.tensor.matmul(px, w1_sb[:, i2, :, c, :],
                                 xt_sb[:, 2 * i2:2 * i2 + 2, bsl],
                                 start=(i2 == 0), stop=(i2 == KP - 1), perf_mode=DR)
            eng = nc.vector if GATING_ENG[c] == "dve" else nc.gpsimd
            eng.tensor_mul(gxvcs[:, i, j, :], px, grep8)

        def out_phase(blk, state):
            """Second contraction + residual epilogue + store for one block."""
            gxvcs, r4, expT_bf = state
            out_t = outp.tile([P, NSUB, D], BF16, tag="o")
            tmp_t = outp.tile([P, NSUB, D], BF16, tag="tmp")
            for s in range(NSUB):
                ssl = slice(s * P, (s + 1) * P)
                po = ps_out.tile([P, D], F32, tag="po")
                for i2 in range(KP):
                    nc.tensor.matmul(po, gxvcs[:, i2, :, ssl], ut_sb[:, i2],
                                     start=(i2 == 0),
                                     stop=(zero_b and i2 == KP - 1), perf_mode=DR)
                if not zero_b:
                    nc.tensor.matmul(po, expT_bf[:, ssl], bexp_sb,
                                     start=False, stop=True)
                x0s = x0_sb[:, blk, s, :]
                rs = r4[:, s:s + 1]
                mode = STT_MODE[s]
                if mode == "act2x":
                    # ACT drains psum to bf16; DVE does the cheap 16-bit
                    # (po*r)*x0 fused multiply (Pool lacks TensorScalarPtr).
                    po_bf = pwork.tile([P, D], BF16, tag="pobf")
                    nc.scalar.copy(po_bf, po)
                    nc.vector.scalar_tensor_tensor(
                        tmp_t[:, s, :], po_bf, rs, x0s,
                        op0=mybir.AluOpType.mult, op1=mybir.AluOpType.mult)
                else:
                    nc.vector.scalar_tensor_tensor(
                        tmp_t[:, s, :], po, rs, x0s,
                        op0=mybir.AluOpType.mult, op1=mybir.AluOpType.mult)
                nc.gpsimd.tensor_add(out_t[:, s, :], tmp_t[:, s, :],
                                     xr_sb[:, blk, s, :])
            nc.sync.dma_start(out_d[:, blk], out_t)

        def with_bf(gate):
            expT, r4 = gate
            expT_bf = None
            if not zero_b:
                expT_bf = gwork.tile([E, BLOCK], BF16, tag="expT_bf")
                nc.scalar.copy(expT_bf, expT)
            return expT, r4, expT_bf

        prev = None
        gate = with_bf(gate_head(0))
        for blk in range(NBLK):
            expT, r4, expT_bf = gate
            gxvcs = gx.tile([P, KP, 2, BLOCK], FP8, tag="gx")
            chunk_phase(blk, expT, gxvcs, 0)
            chunk_phase(blk, expT, gxvcs, 1)
            if blk + 1 < NBLK:
                gate = with_bf(gate_head(blk + 1))
            if prev is not None:
                out_phase(blk - 1, prev)
            chunk_phase(blk, expT, gxvcs, 2)
            chunk_phase(blk, expT, gxvcs, 3)
            prev = (gxvcs, r4, expT_bf)
        out_phase(NBLK - 1, prev)


def _split_excess_waits(nc: bass.Bass, cap: int = 1) -> None:
    """Walrus's per-instruction sync encoders take few wait slots (the TT
    struct rejects 2+). Move extra semaphore waits onto preceding NoOps on
    the same engine; engine program order preserves the semantics."""
    counter = [0]
    for f in nc.m.functions:
        for blk in f.blocks:
            il = blk.instructions
            out = []
            changed = False
            for ins in il:
                si = ins.sync_info
                if si is not None and len(si.on_wait) > cap:
                    extra = list(si.on_wait[:-cap]) if cap else list(si.on_wait)
                    keep = list(si.on_wait[-cap:]) if cap else []
                    for w in extra:
                        nop = mybir.InstNoOp(name=f"NOPW-{counter[0]}")
                        counter[0] += 1
                        nop.engine = ins.engine
                        nop.sync_info = mybir.SyncInfo(on_wait=[w], on_update=[])
                        nc.register_instruction(nop)
                        out.append(nop)
                    ins.sync_info = mybir.SyncInfo(on_wait=keep,
                                                   on_update=list(si.on_update))
                    changed = True
                out.append(ins)
            if changed:
                blk.instructions = out


def build_module(zero_b: bool = True) -> bass.Bass:
    nc = bass.Bass("TRN2", target_bir_lowering=False, debug=False)
    xt_d = nc.dram_tensor("xt8", [P, KC, BL], FP8, kind="ExternalInput").ap()
    xr_d = nc.dram_tensor("xr", [P, NBLK, NSUB, D], BF16, kind="ExternalInput").ap()
    x0_d = nc.dram_tensor("x0q", [P, NBLK, NSUB, D], FP8, kind="ExternalInput").ap()
    w1_d = nc.dram_tensor("w1", [P, KP, 2, NCH, P], FP8, kind="ExternalInput").ap()
    wg_d = nc.dram_tensor("wg", [P, KP, 2, EPAD], FP8, kind="ExternalInput").ap()
    ut_d = nc.dram_tensor("ut", [P, KP, 2, D], FP8, kind="ExternalInput").ap()
    sel_d = nc.dram_tensor("sel", [E, NCH, P], FP8, kind="ExternalInput").ap()
    bias_d = nc.dram_tensor("gbias", [E, 1], F32, kind="ExternalInput").ap()
    bexp_d = nc.dram_tensor("bexp", [E, D], BF16, kind="ExternalInput").ap()
    out_d = nc.dram_tensor("out", [P, NBLK, NSUB, D], BF16,
                           kind="ExternalOutput").ap()
    with tile.TileContext(nc) as tc:
        _kernel_body(tc, out_d, xt_d, xr_d, x0_d, w1_d, wg_d, ut_d, sel_d,
                     bias_d, bexp_d, zero_b)
    _split_excess_waits(nc)
    return nc


_NC_CACHE: dict = {}


def _get_module(zero_b: bool = True) -> bass.Bass:
    if zero_b not in _NC_CACHE:
        _NC_CACHE[zero_b] = build_module(zero_b)
    return _NC_CACHE[zero_b]


def make_in_maps(x0, x, U, V, C, b, Wg, bg):
    x0 = np.asarray(x0, dtype=np.float32)
    x = np.asarray(x, dtype=np.float32)
    U = np.asarray(U, dtype=np.float32)
    V = np.asarray(V, dtype=np.float32)
    C = np.asarray(C, dtype=np.float32)
    b = np.asarray(b, dtype=np.float32)
    Wg = np.asarray(Wg, dtype=np.float32)
    bg = np.asarray(bg, dtype=np.float32)

    # W1[e] = V[e].T @ C[e] -> [D, ES]; packed for DoubleRow lhsT slices
    w1t = np.einsum("erd,ers->eds", V, C).transpose(1, 0, 2).reshape(D, ES)
    w1 = (w1t * SC_W1).reshape(KP, 2, P, NCH, P).transpose(2, 0, 1, 3, 4)
    w1 = np.ascontiguousarray(w1).astype(NPFP8)
    wg_p = (Wg.T * SC_WG).reshape(KP, 2, P, E).transpose(2, 0, 1, 3)
    wg = np.zeros((P, KP, 2, EPAD), np.float32)
    wg[..., :E] = wg_p
    wg = wg.astype(NPFP8)
    # Upk[e*R+s, d] = U[e,d,s]; packed for DoubleRow rhs slices
    ut = (U.transpose(0, 2, 1).reshape(ES, D) * SC_UT)
    ut = ut.reshape(KP, 2, P, D).transpose(2, 0, 1, 3)
    ut = np.ascontiguousarray(ut).astype(NPFP8)
    # replicate matrices: chunk c covers experts 2c (ranks 0-63), 2c+1 (64-127)
    sel = np.zeros((E, NCH, P), np.float32)
    for c in range(NCH):
        sel[2 * c, c, :R] = 1.0
        sel[2 * c + 1, c, R:] = 1.0
    sel = sel.astype(NPFP8)
    gbias = (bg.astype(np.float64) + EXP_BIAS).astype(np.float32).reshape(E, 1)
    bexp = (b * SC_PS).astype(NPBF16)

    x0_bf = x0.astype(NPFP8)
    x_bf = x.astype(NPBF16)
    xt_all = np.ascontiguousarray(x.T).astype(NPFP8)  # [D, B]

    in_maps = []
    for core in range(NCORES):
        sl = slice(core * BL, (core + 1) * BL)
        xt8 = np.ascontiguousarray(
            xt_all[:, sl].reshape(KC, P, BL).transpose(1, 0, 2))
        xr = np.ascontiguousarray(
            x_bf[sl].reshape(NBLK, NSUB, P, D).transpose(2, 0, 1, 3))
        x0q = np.ascontiguousarray(
            x0_bf[sl].reshape(NBLK, NSUB, P, D).transpose(2, 0, 1, 3))
        in_maps.append({
            "xt8": xt8, "xr": xr, "x0q": x0q,
            "w1": w1, "wg": wg, "ut": ut, "sel": sel, "gbias": gbias,
            "bexp": bexp,
        })
    return in_maps


def kernel(x0, x, U, V, C, b, Wg, bg, _trace=False, _trace_kwargs=None):
    from concourse.bass_utils import run_bass_kernel_spmd
    nc = _get_module(not np.any(np.asarray(b)))
    in_maps = make_in_maps(x0, x, U, V, C, b, Wg, bg)
    res = run_bass_kernel_spmd(nc, in_maps, list(range(NCORES)),
                               trace=_trace, **(_trace_kwargs or {}))
    outs = []
    for c in range(NCORES):
        o = np.asarray(res.results[c]["out"])  # [P, NBLK, NSUB, D] bf16
        outs.append(o.transpose(1, 2, 0, 3).reshape(BL, D))
    out = np.concatenate(outs, axis=0).astype(np.float32)
    if _trace:
        return out, res
    return out


if __name__ == "__main__":
    rng = np.random.default_rng(0)
    ins = {
        "x0": rng.standard_normal((B, D), dtype=np.float32),
        "x": rng.standard_normal((B, D), dtype=np.float32),
        "U": (rng.standard_normal((E, D, R)) * 0.02).astype(np.float32),
        "V": (rng.standard_normal((E, R, D)) * 0.02).astype(np.float32),
        "C": (rng.standard_normal((E, R, R)) * 0.02).astype(np.float32),
        "b": np.zeros((E, D), np.float32),
        "Wg": (rng.standard_normal((E, D)) * 0.02).astype(np.float32),
        "bg": np.zeros((E,), np.float32),
    }
    out = kernel(**ins)
    print("out", out.shape, out.dtype)
